# revision 1
# baseline (speedup 1.0000x reference)
"""3-layer GNN message passing (DeepEncoder) on 8 Trainium2 NeuronCores.

Strategy (graph/1D row parallelism):
  - Nodes are row-sharded across the 8 cores; edges are partitioned by src.
  - Per layer: u = x @ W computed on the owning core, AllGather(u) so every
    core holds the full [N, D] activation table in HBM, then each core runs
    its shard of the spmm y[r] = sum_{src[e]==r} val[e] * u[dst[e]].
  - The spmm gather uses the gpsimd dma_gather MoE primitive (int16 indices,
    window-local, 4 windows of <=32768 rows), and the segment-sum uses the
    tensor engine: per 128-edge tile a host-built val-folded one-hot scatter
    matrix S contracts the edge dimension directly into PSUM.
  - Layers 1-2 produce yT tiles (feature dim on partitions) so the next
    W matmul needs no transposes; layer 3 emits row-major fp32 output.

Everything is fp16 on-chip except PSUM accumulation (fp32) and the final
output (fp32).  Measured end-to-end scale-relative error ~2e-4.
"""
import sys

sys.path.insert(0, "/opt/trn_rl_repo")

import numpy as np

from concourse import bacc, mybir
import concourse.tile as tile
from concourse.bass_utils import run_bass_kernel_spmd
from concourse.vector_clock import ScopedClock
import concourse.bass as bass

NCORES = 8
P = 128
NW = 4  # dst windows (window size must stay <= 32768 for int16 indices)


class PatchedTileContext(tile.TileContext):
    """Walrus CoreV3 codegen rejects >2 sync waits on InstDrain; park the
    kernel-tail waits on individual InstNoOps ahead of a bare drain."""

    def _drain_and_barrier(self, tick_clock, wait_clock):
        nop_collect = self.nc.sync.nop()
        wait_clock.add_sem_waits(
            nop_collect.ins, ScopedClock({None: tick_clock.global_clock})
        )
        si = nop_collect.ins.sync_info
        waits = list(si.on_wait) if si is not None and si.on_wait else []
        if si is not None:
            si.on_wait = waits[:1]
        for w in waits[1:]:
            extra = self.nc.sync.nop()
            extra.ins.sync_info = mybir.SyncInfo(on_wait=[w], on_update=[])
        self.nc.sync.drain()
        self.nc.all_engine_barrier()
        assert self.sems is not None
        popped = self.nc._tile_sem_poison_stack.pop()
        assert popped is self._sem_poison
        self.nc.clear_and_free_semaphores(list(self.sems.allocated().values()))
        self.nc.all_engine_barrier()


def _round_up(x, m):
    return (x + m - 1) // m * m


def _prep(feat, edge_src, edge_dst, edge_val):
    """Shard + pack edges.  Returns (per_core_tensors, meta)."""
    N, IN_FEAT = feat.shape
    B = N // NCORES
    R = _round_up(B, P)
    N_pad = NCORES * R
    nrt = R // P
    WROWS = N_pad // NW
    assert WROWS <= 32768 and N_pad % NW == 0

    c_of = edge_src // B
    r_loc = edge_src - c_of * B
    rt_of = r_loc // P
    g_dst = (edge_dst // B) * R + (edge_dst % B)
    w_of = g_dst // WROWS
    lidx = (g_dst - w_of * WROWS).astype(np.int64)

    key = (c_of.astype(np.int64) * nrt + rt_of) * NW + w_of
    counts = np.bincount(key, minlength=NCORES * nrt * NW).reshape(NCORES, nrt, NW)
    k_w = np.array([int(_round_up(max(int(counts[:, :, w].max()), 1), P) // P)
                    for w in range(NW)])
    T = int(k_w.sum())
    cum_t = np.concatenate([[0], np.cumsum(k_w)])
    cols_w = k_w * 8
    cum_c = np.concatenate([[0], np.cumsum(cols_w)])
    cols_rt = int(cols_w.sum())

    order = np.lexsort((g_dst, w_of, rt_of, c_of))
    skey = key[order]
    uniq, start_idx = np.unique(skey, return_index=True)
    group_start = np.zeros(NCORES * nrt * NW, dtype=np.int64)
    group_start[uniq] = start_idx
    rank = np.arange(len(order)) - group_start[skey]

    rt_s = rt_of[order]
    w_s = w_of[order]
    c_s = c_of[order]
    slot = rt_s * (T * P) + cum_t[w_s] * P + rank
    lidx_s = lidx[order]
    val_s = edge_val[order].astype(np.float16)
    row_s = (r_loc[order] - rt_s * P).astype(np.int64)

    per_core = []
    for c in range(NCORES):
        m = c_s == c
        sl, li, va, ro = slot[m], lidx_s[m], val_s[m], row_s[m]
        S = np.zeros((nrt * T, P, P), dtype=np.float16)
        S[sl // P, sl % P, ro] = va
        Smat = np.ascontiguousarray(S.transpose(1, 0, 2)).reshape(P, nrt * T * P)
        L = np.zeros(nrt * T * P, dtype=np.int16)
        L[sl] = li.astype(np.int16)
        gidx = np.zeros((16, nrt * cols_rt), dtype=np.int16)
        for rt in range(nrt):
            for w in range(NW):
                v = L[rt * T * P + cum_t[w] * P: rt * T * P + cum_t[w + 1] * P]
                gidx[:, rt * cols_rt + cum_c[w]: rt * cols_rt + cum_c[w + 1]] = (
                    v.reshape(-1, 16).T
                )
        fT = np.zeros((IN_FEAT, R), dtype=np.float16)
        fT[:, :B] = feat[c * B:(c + 1) * B].T.astype(np.float16)
        per_core.append({
            "featT": fT,
            "Smat": Smat,
            "gidx": np.tile(gidx, (8, 1)),
        })
    meta = dict(N=N, B=B, R=R, N_pad=N_pad, nrt=nrt, WROWS=WROWS,
                k_w=k_w.tolist(), T=T, cum_t=cum_t.tolist(),
                cum_c=cum_c.tolist(), cols_rt=cols_rt, IN_FEAT=IN_FEAT)
    return per_core, meta


def _build(meta, hid, out_feat):
    R, nrt, T = meta["R"], meta["nrt"], meta["T"]
    N_pad, WROWS = meta["N_pad"], meta["WROWS"]
    k_w, cum_t = meta["k_w"], meta["cum_t"]
    cum_c, cols_rt = meta["cum_c"], meta["cols_rt"]
    IN_FEAT = meta["IN_FEAT"]
    KB_IN = IN_FEAT // P
    KB_H = hid // P
    f16, f32 = mybir.dt.float16, mybir.dt.float32

    nc = bacc.Bacc()
    featT = nc.dram_tensor("featT", [IN_FEAT, R], f16, kind="ExternalInput")
    Smat = nc.dram_tensor("Smat", [P, nrt * T * P], f16, kind="ExternalInput")
    gidx = nc.dram_tensor("gidx", [P, nrt * cols_rt], mybir.dt.int16,
                          kind="ExternalInput")
    W0 = nc.dram_tensor("W0", [IN_FEAT, hid], f16, kind="ExternalInput")
    W1 = nc.dram_tensor("W1", [hid, hid], f16, kind="ExternalInput")
    W2 = nc.dram_tensor("W2", [hid, out_feat], f16, kind="ExternalInput")
    out = nc.dram_tensor("out", [R, out_feat], f32, kind="ExternalOutput")

    ag_in0 = nc.dram_tensor("ag_in0", [R, hid], f16)
    ag_out0 = nc.dram_tensor("ag_out0", [N_pad, hid], f16, addr_space="Shared")
    ag_in1 = nc.dram_tensor("ag_in1", [R, hid], f16)
    ag_out1 = nc.dram_tensor("ag_out1", [N_pad, hid], f16, addr_space="Shared")
    ag_in2 = nc.dram_tensor("ag_in2", [R, out_feat], f16)
    ag_out2 = nc.dram_tensor("ag_out2", [N_pad, out_feat], f16, addr_space="Shared")

    rg = [list(range(NCORES))]

    with PatchedTileContext(nc) as tc:
        with tc.tile_pool(name="const", bufs=1) as cpool:
            w0_t = [cpool.tile([P, hid], f16, tag=f"w0_{k}", name=f"w0_{k}")
                    for k in range(KB_IN)]
            for k in range(KB_IN):
                nc.sync.dma_start(out=w0_t[k][:], in_=W0[k * P:(k + 1) * P, :])
            w1_t = [cpool.tile([P, hid], f16, tag=f"w1_{k}", name=f"w1_{k}")
                    for k in range(KB_H)]
            for k in range(KB_H):
                nc.sync.dma_start(out=w1_t[k][:], in_=W1[k * P:(k + 1) * P, :])
            w2_t = [cpool.tile([P, out_feat], f16, tag=f"w2_{k}", name=f"w2_{k}")
                    for k in range(KB_H)]
            for k in range(KB_H):
                nc.sync.dma_start(out=w2_t[k][:], in_=W2[k * P:(k + 1) * P, :])

            # ---- phase A: u0 = feat @ W0 ----
            with (
                tc.tile_pool(name="pa_sb", bufs=3) as sb,
                tc.tile_pool(name="pa_ps", bufs=2, space="PSUM") as ps,
            ):
                for rt in range(nrt):
                    f_t = sb.tile([P, KB_IN * P], f16, tag="f_t")
                    for k in range(KB_IN):
                        nc.sync.dma_start(
                            out=f_t[:, k * P:(k + 1) * P],
                            in_=featT[k * P:(k + 1) * P, rt * P:(rt + 1) * P],
                        )
                    z_ps = ps.tile([P, hid], f32, tag="z_ps")
                    for k in range(KB_IN):
                        nc.tensor.matmul(
                            z_ps[:], lhsT=f_t[:, k * P:(k + 1) * P], rhs=w0_t[k][:],
                            start=(k == 0), stop=(k == KB_IN - 1),
                        )
                    z_sb = sb.tile([P, hid], f16, tag="z_sb")
                    nc.vector.tensor_copy(z_sb[:], z_ps[:])
                    nc.sync.dma_start(out=ag_in0[rt * P:(rt + 1) * P, :], in_=z_sb[:])
            nc.gpsimd.collective_compute(
                "AllGather", mybir.AluOpType.bypass, ins=[ag_in0.ap().opt()],
                outs=[ag_out0.ap().opt()], replica_groups=rg,
            )

            def spmm_layer(u, d_in, w_t, ag_dst, d_out):
                """ag_dst[rows] = relu(spmm(u))[rows] @ W  for this core's rows."""
                db = d_in // P
                with (
                    tc.tile_pool(name="sp_sb", bufs=2) as sb,
                    tc.tile_pool(name="sp_ps", bufs=2, space="PSUM") as ps,
                    tc.tile_pool(name="sp_ps2", bufs=2, space="PSUM") as ps2,
                ):
                    for rt in range(nrt):
                        idx_t = sb.tile([P, cols_rt], mybir.dt.int16, tag="idx_t")
                        nc.sync.dma_start(
                            out=idx_t[:],
                            in_=gidx[:, rt * cols_rt:(rt + 1) * cols_rt],
                        )
                        s_t = sb.tile([P, T * P], f16, tag="s_t")
                        nc.sync.dma_start(
                            out=s_t[:], in_=Smat[:, rt * T * P:(rt + 1) * T * P]
                        )
                        g_t = sb.tile([P, T, d_in], f16, tag="g_t")
                        for w in range(NW):
                            nc.gpsimd.dma_gather(
                                out_ap=g_t[:, cum_t[w]:cum_t[w + 1], :],
                                in_ap=u[w * WROWS:(w + 1) * WROWS, :],
                                idxs_ap=idx_t[:, cum_c[w]:cum_c[w + 1]],
                                num_idxs=k_w[w] * P,
                                num_idxs_reg=k_w[w] * P,
                                elem_size=d_in,
                                single_packet=(k_w[w] * P <= 1024),
                            )
                        yT_ps = [ps.tile([P, P], f32, tag=f"yT{b}", name=f"yT{b}")
                                 for b in range(db)]
                        for t in range(T):
                            for b in range(db):
                                nc.tensor.matmul(
                                    yT_ps[b][:],
                                    lhsT=g_t[:, t, b * P:(b + 1) * P],
                                    rhs=s_t[:, t * P:(t + 1) * P],
                                    start=(t == 0), stop=(t == T - 1),
                                )
                        z_ps = ps2.tile([P, d_out], f32, tag="z_ps")
                        for b in range(db):
                            yT_sb = sb.tile([P, P], f16, tag=f"yT_sb{b}",
                                            name=f"yT_sb{b}")
                            nc.vector.tensor_scalar_max(yT_sb[:], yT_ps[b][:], 0.0)
                            nc.tensor.matmul(
                                z_ps[:], lhsT=yT_sb[:], rhs=w_t[b][:],
                                start=(b == 0), stop=(b == db - 1),
                            )
                        z_sb = sb.tile([P, d_out], f16, tag="z_sb")
                        nc.vector.tensor_copy(z_sb[:], z_ps[:])
                        nc.sync.dma_start(
                            out=ag_dst[rt * P:(rt + 1) * P, :], in_=z_sb[:]
                        )

            spmm_layer(ag_out0, hid, w1_t, ag_in1, hid)
            nc.gpsimd.collective_compute(
                "AllGather", mybir.AluOpType.bypass, ins=[ag_in1.ap().opt()],
                outs=[ag_out1.ap().opt()], replica_groups=rg,
            )
            spmm_layer(ag_out1, hid, w2_t, ag_in2, out_feat)
            nc.gpsimd.collective_compute(
                "AllGather", mybir.AluOpType.bypass, ins=[ag_in2.ap().opt()],
                outs=[ag_out2.ap().opt()], replica_groups=rg,
            )

            # ---- layer 3: out = spmm(u2), row-major fp32, no relu ----
            with (
                tc.tile_pool(name="p3_sb", bufs=2) as sb,
                tc.tile_pool(name="p3_ps", bufs=2, space="PSUM") as ps,
            ):
                for rt in range(nrt):
                    idx_t = sb.tile([P, cols_rt], mybir.dt.int16, tag="idx_t")
                    nc.sync.dma_start(
                        out=idx_t[:], in_=gidx[:, rt * cols_rt:(rt + 1) * cols_rt]
                    )
                    s_t = sb.tile([P, T * P], f16, tag="s_t")
                    nc.sync.dma_start(
                        out=s_t[:], in_=Smat[:, rt * T * P:(rt + 1) * T * P]
                    )
                    g_t = sb.tile([P, T, out_feat], f16, tag="g_t")
                    for w in range(NW):
                        nc.gpsimd.dma_gather(
                            out_ap=g_t[:, cum_t[w]:cum_t[w + 1], :],
                            in_ap=ag_out2[w * WROWS:(w + 1) * WROWS, :],
                            idxs_ap=idx_t[:, cum_c[w]:cum_c[w + 1]],
                            num_idxs=k_w[w] * P,
                            num_idxs_reg=k_w[w] * P,
                            elem_size=out_feat,
                            single_packet=(k_w[w] * P <= 1024),
                        )
                    o_ps = ps.tile([P, out_feat], f32, tag="o_ps")
                    for t in range(T):
                        nc.tensor.matmul(
                            o_ps[:], lhsT=s_t[:, t * P:(t + 1) * P],
                            rhs=g_t[:, t, :],
                            start=(t == 0), stop=(t == T - 1),
                        )
                    o_sb = sb.tile([P, out_feat], f32, tag="o_sb")
                    nc.vector.tensor_copy(o_sb[:], o_ps[:])
                    nc.sync.dma_start(out=out[rt * P:(rt + 1) * P, :], in_=o_sb[:])
    nc.compile()
    return nc


_CACHE = {}


def kernel(feat, edge_src, edge_dst, edge_val, W0, W1, W2, trace=False):
    feat = np.asarray(feat, dtype=np.float32)
    edge_src = np.asarray(edge_src, dtype=np.int32)
    edge_dst = np.asarray(edge_dst, dtype=np.int32)
    edge_val = np.asarray(edge_val, dtype=np.float32)
    W0 = np.asarray(W0, dtype=np.float32)
    W1 = np.asarray(W1, dtype=np.float32)
    W2 = np.asarray(W2, dtype=np.float32)

    per_core, meta = _prep(feat, edge_src, edge_dst, edge_val)
    hid, out_feat = W0.shape[1], W2.shape[1]
    key = (meta["N"], meta["T"], tuple(meta["k_w"]), hid, out_feat)
    if key not in _CACHE:
        _CACHE[key] = _build(meta, hid, out_feat)
    nc = _CACHE[key]

    w_common = {
        "W0": W0.astype(np.float16),
        "W1": W1.astype(np.float16),
        "W2": W2.astype(np.float16),
    }
    in_maps = [{**pc, **w_common} for pc in per_core]
    res = run_bass_kernel_spmd(nc, in_maps, core_ids=list(range(NCORES)),
                               trace=trace)
    B, N = meta["B"], meta["N"]
    full = np.concatenate([r["out"][:B] for r in res.results], axis=0)[:N]
    if trace:
        kernel._last_result = res
    return full.astype(np.float32)

